# revision 14
# baseline (speedup 1.0000x reference)
"""GATv2 classifier kernel for Trainium2, 8-core SPMD.

Strategy (v2):
  - Nodes dealt round-robin by descending in-degree across 8 cores; edges
    partitioned by destination so segment-softmax stays core-local. Every
    core redundantly computes the full xl = x@Wl table (cheap matmul) into
    its own DRAM tables (lo/hi split for int16 gather indices).
  - Self-loops are NOT gathered: their contribution is computed per bucket
    from x@(Wl+Wr) directly (dense matmuls on resident xT chunks).
  - Gathers use prepare_only descriptor generation + trigger_dma: the Pool
    engine streams descriptor generation from t=0 (overlapping the table
    build) and per-bucket triggers fire the DMAs two buckets ahead.
  - Padding slots index table row 0, which holds -512*att ("poison"):
    e = att.lrelu(poison + xr) ~ -300 so exp underflows to exactly 0.
    No mask tensors, no dummy-slot bookkeeping.
  - agg = sum_k p_k * g_k is computed on the TensorEngine by accumulating
    diag(p_k) @ g_k into PSUM (diag built by one DVE tensor_scalar per
    slot), replacing the broadcast-copy + multiply + strided-reduce chain.
    Since agg uses g (not z), no xr correction is needed:
      out = sigmoid((agg @ Wo) / den + (bl + bias) @ Wo + bo)
  - Pass L writes fp16 partials (agg row + den) to DRAM; pass H merges them
    via one gather + an identity matmul into the PSUM accumulator.
  - sigmoid via tanh (same ACT table set as exp/prelu): one table load.
"""

import math
import os
import sys

import numpy as np

if os.path.isdir("/opt/trn_rl_repo") and "/opt/trn_rl_repo" not in sys.path:
    sys.path.insert(0, "/opt/trn_rl_repo")

P = 128
NEG_SLOPE = 0.2
CHUNK = 512          # nodes per phase-1 table-build chunk
PARTW = 128          # fp32 words per partial row (aw, den, pad), 512B
POISON = -512.0


# --------------------------------------------------------------------------
# Host-side planning
# --------------------------------------------------------------------------

def _wrap_idx(grid):
    """[K,128] slot grid -> dma_gather wrapped idx layout [128, K*128/16]."""
    flat = grid.reshape(-1).astype(np.int16)
    return np.tile(flat.reshape(-1, 16).T, (8, 1))


def _plan(x, edge_index, Wl, bl, Wr, br, att, bias, Wo, bo, n_cores=8):
    N, F = x.shape
    assert F == P
    C = n_cores

    src = np.asarray(edge_index[0], dtype=np.int64)
    dst = np.asarray(edge_index[1], dtype=np.int64)
    deg = np.bincount(dst, minlength=N)          # real in-degree, no self

    n_chunks = (N + CHUNK - 1) // CHUNK
    N_pad = n_chunks * CHUNK
    L_lo = min(n_chunks, 63)
    LO = L_lo * CHUNK
    e_order = np.lexsort((src >= LO, dst))       # by dst, lo srcs first
    src_sorted = src[e_order]
    starts = np.concatenate([[0], np.cumsum(deg)]).astype(np.int64)
    lo_cnt = np.bincount(dst, weights=(src < LO).astype(np.float64),
                         minlength=N).astype(np.int64)
    hi_cnt = deg - lo_cnt

    # deal nodes round-robin by descending total degree
    order = np.argsort(-deg, kind="stable")
    npc = (N + C - 1) // C
    NB = (npc + P - 1) // P
    npc_pad = NB * P
    order_pad = np.full(C * npc_pad, -1, dtype=np.int64)
    order_pad[:N] = order
    core_nodes = np.stack([order_pad[c::C] for c in range(C)])  # [C, npc_pad]

    def pass_order(cnt):
        orders = np.zeros((C, npc_pad), dtype=np.int64)
        for c in range(C):
            nodes = core_nodes[c]
            key = np.where(nodes >= 0, cnt[np.maximum(nodes, 0)], -1)
            orders[c] = np.argsort(-key, kind="stable")
        return orders

    ordL = pass_order(lo_cnt)   # positions into core_nodes[c]
    ordH = pass_order(hi_cnt)

    def k_sched(cnt, orders):
        Ks = []
        for b in range(NB):
            m = 0
            for c in range(C):
                nodes = core_nodes[c][orders[c][b * P:(b + 1) * P]]
                ok = nodes >= 0
                if ok.any():
                    m = max(m, int(cnt[nodes[ok]].max()))
            Ks.append(m)
        return Ks

    KsL = k_sched(lo_cnt, ordL)
    KsH = k_sched(hi_cnt, ordH)

    def offs(Ks):
        so, s = [], 0
        for K in Ks:
            so.append(s)
            s += K * 8          # int16 idx columns per bucket
        return so, max(s, 8)

    soL, StotL16 = offs(KsL)
    soH, StotH16 = offs(KsH)

    xT16 = np.asarray(x, dtype=np.float16).T            # [128, N]

    idxL = np.zeros((C, P, StotL16), dtype=np.int16)
    idxH = np.zeros((C, P, StotH16), dtype=np.int16)
    xT_L = np.zeros((C, P, npc_pad), dtype=np.float16)
    xT_H = np.zeros((C, P, npc_pad), dtype=np.float16)
    merge_idx = np.zeros((C, P, npc_pad // 16), dtype=np.int16)

    for c in range(C):
        nodes = core_nodes[c]
        posL_of = np.empty(npc_pad, dtype=np.int64)
        posL_of[ordL[c]] = np.arange(npc_pad)

        for (idx_a, xt_a, Ks, so_a, orders, cnt, base, is_lo) in (
            (idxL, xT_L, KsL, soL, ordL, lo_cnt, 0, True),
            (idxH, xT_H, KsH, soH, ordH, hi_cnt, LO, False),
        ):
            o = orders[c]
            nds = nodes[o]                      # node id per position
            ok = nds >= 0
            xt_a[c][:, ok] = xT16[:, nds[ok]]
            cnts = np.where(ok, cnt[np.maximum(nds, 0)], 0)
            seg0 = starts[np.maximum(nds, 0)] + (0 if is_lo else
                                                 lo_cnt[np.maximum(nds, 0)])
            for b in range(NB):
                K = Ks[b]
                if K == 0:
                    continue
                sl = slice(b * P, (b + 1) * P)
                db = cnts[sl]
                kk = np.arange(K)[:, None]                     # [K, 128]
                valid = kk < db[None, :]
                pos = seg0[sl][None, :] + kk
                srcg = np.where(valid,
                                src_sorted[np.minimum(pos, len(src_sorted) - 1)], 0)
                rel = np.where(valid, srcg - base + 1, 0)
                idx_a[c][:, so_a[b]:so_a[b] + K * 8] = _wrap_idx(rel)
            if not is_lo:
                merge_idx[c] = _wrap_idx(posL_of[o].reshape(npc_pad // P, P))

    xT_full = np.zeros((P, N_pad), dtype=np.float16)
    xT_full[:, :N] = xT16

    wl = np.asarray(Wl, dtype=np.float16)
    wr = np.asarray(Wr, dtype=np.float16)
    wlr = (np.asarray(Wl, dtype=np.float64)
           + np.asarray(Wr, dtype=np.float64)).astype(np.float16)
    blbr = (np.asarray(bl, dtype=np.float64)
            + np.asarray(br, dtype=np.float64)).astype(np.float16).reshape(1, P)
    att16 = np.asarray(att, dtype=np.float16)
    att_rep = np.tile(att16[None, None, :], (P, 4, 1))           # [128,4,128]
    i_rep = np.tile(np.eye(P, dtype=np.float16)[:, None, :], (1, 4, 1))
    wo_rep = np.tile(np.asarray(Wo, dtype=np.float16)[:, 0][None, :], (P, 1))
    poison = (POISON * np.asarray(att, dtype=np.float64)).astype(
        np.float16).reshape(1, P)
    bo_eff = float(np.asarray(bo).reshape(-1)[0] +
                   (np.asarray(bl, dtype=np.float64)
                    + np.asarray(bias, dtype=np.float64))
                   @ np.asarray(Wo, dtype=np.float64)[:, 0])

    cfg = dict(N=N, C=C, NB=NB, npc_pad=npc_pad,
               KsL=KsL, KsH=KsH, soL=soL, soH=soH,
               StotL16=StotL16, StotH16=StotH16,
               n_chunks=n_chunks, N_pad=N_pad, L_lo=L_lo, LO=LO,
               lo_rows=LO + 1, hi_rows=max(N_pad - LO, 1) + 1,
               bo_eff=bo_eff)

    in_maps = []
    for c in range(C):
        in_maps.append({
            "xT_full": xT_full,
            "xT_L": np.ascontiguousarray(xT_L[c]),
            "xT_H": np.ascontiguousarray(xT_H[c]),
            "idx_L": np.ascontiguousarray(idxL[c]),
            "idx_H": np.ascontiguousarray(idxH[c]),
            "merge_idx": np.ascontiguousarray(merge_idx[c]),
            "wl": wl, "wr": wr, "wlr": wlr, "blbr": blbr,
            "att_rep": att_rep, "i_rep": i_rep, "wo_rep": wo_rep,
            "poison": poison,
        })
    out_nodes = np.stack([core_nodes[c][ordH[c]] for c in range(C)])
    return cfg, in_maps, out_nodes


# --------------------------------------------------------------------------
# Device program
# --------------------------------------------------------------------------

def _build(cfg, lrelu_act=True, debug=False):
    import concourse.bass as bass
    import concourse.bacc as bacc
    import concourse.tile as tile
    from concourse import mybir

    f16, f32, i16 = mybir.dt.float16, mybir.dt.float32, mybir.dt.int16
    AT = mybir.ActivationFunctionType
    OP = mybir.AluOpType
    AX = mybir.AxisListType

    NB = cfg["NB"]
    n_chunks, L_lo = cfg["n_chunks"], cfg["L_lo"]
    npc_pad = cfg["npc_pad"]
    KsL, KsH = cfg["KsL"], cfg["KsH"]
    soL, soH = cfg["soL"], cfg["soH"]
    NQ = 4

    nc = bacc.Bacc("TRN2", target_bir_lowering=False, debug=debug,
                   num_devices=cfg["C"], num_swdge_queues=NQ)

    xT_full = nc.dram_tensor("xT_full", [P, cfg["N_pad"]], f16, kind="ExternalInput")
    xT_L_d = nc.dram_tensor("xT_L", [P, npc_pad], f16, kind="ExternalInput")
    xT_H_d = nc.dram_tensor("xT_H", [P, npc_pad], f16, kind="ExternalInput")
    idx_L_d = nc.dram_tensor("idx_L", [P, cfg["StotL16"]], i16, kind="ExternalInput")
    idx_H_d = nc.dram_tensor("idx_H", [P, cfg["StotH16"]], i16, kind="ExternalInput")
    merge_d = nc.dram_tensor("merge_idx", [P, npc_pad // 16], i16, kind="ExternalInput")
    wl_d = nc.dram_tensor("wl", [P, P], f16, kind="ExternalInput")
    wr_d = nc.dram_tensor("wr", [P, P], f16, kind="ExternalInput")
    wlr_d = nc.dram_tensor("wlr", [P, P], f16, kind="ExternalInput")
    blbr_d = nc.dram_tensor("blbr", [1, P], f16, kind="ExternalInput")
    attr_d = nc.dram_tensor("att_rep", [P, 4, P], f16, kind="ExternalInput")
    irep_d = nc.dram_tensor("i_rep", [P, 4, P], f16, kind="ExternalInput")
    wo_d = nc.dram_tensor("wo_rep", [P, P], f16, kind="ExternalInput")
    poison_d = nc.dram_tensor("poison", [1, P], f16, kind="ExternalInput")
    out_d = nc.dram_tensor("out", [npc_pad, 1], f32, kind="ExternalOutput")

    table_lo = nc.dram_tensor("table_lo", [cfg["lo_rows"], P], f16)
    table_hi = nc.dram_tensor("table_hi", [cfg["hi_rows"], P], f16)
    partial = nc.dram_tensor("partial", [npc_pad, PARTW], f32)

    def bc(ap, pattern):
        return bass.AP(tensor=ap.tensor, offset=ap.offset,
                       ap=[list(ap.ap[0])] + [list(p) for p in pattern])

    with tile.TileContext(nc) as tc:
        with tc.tile_pool(name="const", bufs=1) as cp:
            wl_sb = cp.tile([P, P], f16, tag="wl")
            wr_sb = cp.tile([P, P], f16, tag="wr")
            wlr_sb = cp.tile([P, P], f16, tag="wlr")
            blbr_sb = cp.tile([1, P], f16, tag="blbr")
            att_sb = cp.tile([P, 4, P], f16, tag="attr")
            irep_sb = cp.tile([P, 4, P], f16, tag="irep")
            wo_sb = cp.tile([P, P], f16, tag="wo")
            poison_sb = cp.tile([1, P], f16, tag="poison")
            idxL_sb = cp.tile([P, cfg["StotL16"]], i16, tag="idxL")
            idxH_sb = cp.tile([P, cfg["StotH16"]], i16, tag="idxH")
            merge_sb = cp.tile([P, npc_pad // 16], i16, tag="mergei")
            xT_L_sb = cp.tile([P, npc_pad], f16, tag="xTL")
            xT_H_sb = cp.tile([P, npc_pad], f16, tag="xTH")
            xrT_L = cp.tile([P, NB, P], f16, tag="xrTL")
            xrT_H = cp.tile([P, NB, P], f16, tag="xrTH")
            part_sb = cp.tile([P, NB, PARTW], f32, tag="part")
            ones1 = cp.tile([1, P], f16, tag="ones1")
            tcol = cp.tile([P, NB], f32, tag="tcol")
            out_sb = cp.tile([P, NB], f32, tag="outsb")

            for t, d in ((wl_sb, wl_d), (wr_sb, wr_d), (wlr_sb, wlr_d),
                         (blbr_sb, blbr_d), (att_sb, attr_d),
                         (irep_sb, irep_d), (wo_sb, wo_d),
                         (poison_sb, poison_d), (idxL_sb, idx_L_d),
                         (idxH_sb, idx_H_d), (merge_sb, merge_d),
                         (xT_L_sb, xT_L_d), (xT_H_sb, xT_H_d)):
                nc.sync.dma_start(out=t, in_=d.ap())
            nc.vector.memset(ones1, 1.0)
            nc.sync.dma_start(out=table_lo.ap()[0:1, :], in_=poison_sb)
            nc.sync.dma_start(out=table_hi.ap()[0:1, :], in_=poison_sb)

            i128 = irep_sb[:, 0, :]

            # ------------- gather emission ------------------------------
            # work items: ("L", b), ("M", 0), ("H", b)
            items = ([("L", b) for b in range(NB)] + [("M", 0)]
                     + [("H", b) for b in range(NB)])
            NW = len(items)
            prep_tiles = {}
            qctr = [0]

            def emit_gather(i, gp):
                kind, b = items[i]
                if kind == "M":
                    q = 0
                    qctr[0] += 1
                    nc.gpsimd.dma_gather(
                        out_ap=part_sb, in_ap=partial.ap(),
                        idxs_ap=merge_sb, num_idxs=npc_pad,
                        num_idxs_reg=npc_pad, elem_size=PARTW,
                        queue_num=q)
                    prep_tiles[i] = part_sb
                    return
                Ks = KsL if kind == "L" else KsH
                K = Ks[b]
                if K == 0:
                    return
                idx_sb = idxL_sb if kind == "L" else idxH_sb
                so = (soL if kind == "L" else soH)[b]
                table = table_lo if kind == "L" else table_hi
                q = 0
                qctr[0] += 1
                g = gp.tile([P, K, P], f16, tag="g")
                nc.gpsimd.dma_gather(
                    out_ap=g, in_ap=table.ap(),
                    idxs_ap=idx_sb[:, so:so + K * 8],
                    num_idxs=K * P, num_idxs_reg=K * P, elem_size=P,
                    queue_num=q)
                prep_tiles[i] = g

            # ------------- phase 1a: xrT chunks (one order) ------------
            def phase1a(xt_sb, xrT, lpp):
                for b in range(NB):
                    ps1 = lpp.tile([P, P], f32, tag="ps1")
                    nc.tensor.matmul(ps1, wr_sb,
                                     xt_sb[:, b * P:(b + 1) * P],
                                     start=True, stop=False)
                    nc.tensor.matmul(ps1, blbr_sb, ones1,
                                     start=False, stop=True)
                    if b % 2 == 0:
                        nc.scalar.copy(xrT[:, b, :], ps1)
                    else:
                        nc.vector.tensor_copy(xrT[:, b, :], ps1)

            # ------------- phase 1b: xl tables (no bias) ---------------
            def phase1b(chunks, xp, pp, cvp):
                for ch in chunks:
                    xt = xp.tile([P, 4, P], f16, tag="xt")
                    nc.sync.dma_start(
                        out=xt, in_=xT_full.ap()[:, ch * CHUNK:(ch + 1) * CHUNK])
                    ps = pp.tile([P, 4, P], f32, tag="pch")
                    for i in range(4):
                        nc.tensor.matmul(ps[:, i, :], xt[:, i, :], wl_sb,
                                         start=True, stop=True)
                    cv = cvp.tile([P, 4, P], f16, tag="cv")
                    if ch % 2 == 0:
                        nc.scalar.copy(cv, ps)
                    else:
                        nc.vector.tensor_copy(cv, ps)
                    if ch < L_lo:
                        r0 = ch * CHUNK + 1
                        dst = table_lo.ap()[r0:r0 + CHUNK, :]
                    else:
                        r0 = ch * CHUNK - cfg["LO"] + 1
                        dst = table_hi.ap()[r0:r0 + CHUNK, :]
                    nc.sync.dma_start(
                        out=dst.rearrange("(i n) f -> n i f", n=P), in_=cv)

            # ------------- per-bucket compute --------------------------
            def bucket_compute(i, lp, zp, sp, dp, aggp, zsp, psp):
                kind, b = items[i]
                if kind == "M":
                    return
                is_l = kind == "L"
                K = (KsL if is_l else KsH)[b]
                g = prep_tiles.get(i)
                xrT = xrT_L if is_l else xrT_H

                pm = None
                if K > 0:
                    lr = lp.tile([P, K, P], f16, tag="lr")
                    for j0 in range(0, K, 4):
                        kc = min(4, K - j0)
                        zb = zp.tile([P, 4, P], f32, tag="zb")
                        nc.tensor.matmul(zb[:, 0:kc, :], i128,
                                         g[:, j0:j0 + kc, :],
                                         start=True, stop=False)
                        nc.tensor.matmul(zb[:, 0:kc, :], xrT[:, b, :],
                                         irep_sb[:, 0:kc, :],
                                         start=False, stop=True)
                        if lrelu_act:
                            nc.scalar.activation(lr[:, j0:j0 + kc, :],
                                                 zb[:, 0:kc, :],
                                                 AT.Prelu, alpha=NEG_SLOPE)
                        else:
                            nc.vector.scalar_tensor_tensor(
                                out=lr[:, j0:j0 + kc, :], in0=zb[:, 0:kc, :],
                                scalar=NEG_SLOPE, in1=zb[:, 0:kc, :],
                                op0=OP.mult, op1=OP.max)
                    att_b = bc(att_sb, [[0, K], [1, P]])
                    nc.vector.tensor_mul(lr, lr, att_b)
                    e_t = sp.tile([P, K], f32, tag="e")
                    nc.vector.reduce_sum(out=e_t, in_=lr, axis=AX.X)
                    pm = sp.tile([P, K], f32, tag="pm")
                    nc.scalar.activation(pm, e_t, AT.Exp)
                    den1 = sp.tile([P, 1], f32, tag="den")
                    nc.vector.reduce_sum(out=den1, in_=pm, axis=AX.X)
                else:
                    den1 = sp.tile([P, 1], f32, tag="den")
                    nc.vector.memset(den1, 0.0)

                if is_l:
                    psb = psp.tile([P, PARTW], f32, tag="psb")
                    nc.vector.memset(psb[:, 2:PARTW], 0.0)
                    if K > 0:
                        agg = aggp.tile([P, P], f32, tag="agg")
                        for k in range(K):
                            D = dp.tile([P, P], f16, tag="D")
                            nc.vector.tensor_scalar(
                                D, i128, pm[:, k:k + 1], None, OP.mult)
                            nc.tensor.matmul(agg, D, g[:, k, :],
                                             start=(k == 0), stop=(k == K - 1))
                        scr = sp.tile([P, P], f32, tag="scr")
                        nc.vector.scalar_tensor_tensor(
                            out=scr, in0=agg, scalar=1.0, in1=wo_sb,
                            op0=OP.mult, op1=OP.mult,
                            accum_out=psb[:, 0:1])
                    else:
                        nc.vector.memset(psb[:, 0:1], 0.0)
                    nc.vector.tensor_copy(psb[:, 1:2], den1)
                    nc.sync.dma_start(
                        out=partial.ap()[b * P:(b + 1) * P, :], in_=psb)
                    return

                # ---- pass H: self loop + scalar partial merge ----
                agg = aggp.tile([P, P], f32, tag="agg")
                if K > 0:
                    for k in range(K):
                        D = dp.tile([P, P], f16, tag="D")
                        nc.vector.tensor_scalar(
                            D, i128, pm[:, k:k + 1], None, OP.mult)
                        nc.tensor.matmul(agg, D, g[:, k, :],
                                         start=(k == 0), stop=False)

                # self-loop terms from resident xT_H
                xcol = xT_H_sb[:, b * P:(b + 1) * P]
                zs = zsp.tile([P, 2, P], f32, tag="zs")
                nc.tensor.matmul(zs[:, 0, :], xcol, wlr_sb,
                                 start=True, stop=False)
                nc.tensor.matmul(zs[:, 0, :], blbr_sb, ones1,
                                 start=False, stop=True)
                nc.tensor.matmul(zs[:, 1, :], xcol, wl_sb,
                                 start=True, stop=True)
                lrs = sp.tile([P, P], f16, tag="lrs")
                if lrelu_act:
                    nc.scalar.activation(lrs, zs[:, 0, :], AT.Prelu,
                                         alpha=NEG_SLOPE)
                else:
                    nc.vector.scalar_tensor_tensor(
                        out=lrs, in0=zs[:, 0, :], scalar=NEG_SLOPE,
                        in1=zs[:, 0, :], op0=OP.mult, op1=OP.max)
                nc.vector.tensor_mul(lrs, lrs, att_sb[:, 0, :])
                es = sp.tile([P, 1], f32, tag="es")
                nc.vector.reduce_sum(out=es, in_=lrs, axis=AX.X)
                ps_ = sp.tile([P, 1], f32, tag="ps_")
                nc.scalar.activation(ps_, es, AT.Exp)
                xls_sb = sp.tile([P, P], f16, tag="xlssb")
                nc.scalar.copy(xls_sb, zs[:, 1, :])
                Ds = dp.tile([P, P], f16, tag="D")
                nc.vector.tensor_scalar(Ds, i128, ps_, None, OP.mult)
                nc.tensor.matmul(agg, Ds, xls_sb,
                                 start=(K == 0), stop=True)

                dent = sp.tile([P, 1], f32, tag="dent")
                nc.vector.tensor_add(dent, den1, ps_)
                dent2 = sp.tile([P, 1], f32, tag="dent2")
                nc.vector.tensor_add(dent2, dent, part_sb[:, b, 1:2])
                rden = sp.tile([P, 1], f32, tag="rden")
                nc.vector.reciprocal(rden, dent2)
                scr = sp.tile([P, P], f32, tag="scr")
                aw = sp.tile([P, 1], f32, tag="aw")
                nc.vector.scalar_tensor_tensor(
                    out=scr, in0=agg, scalar=1.0, in1=wo_sb,
                    op0=OP.mult, op1=OP.mult, accum_out=aw)
                aw2 = sp.tile([P, 1], f32, tag="aw2")
                nc.vector.tensor_add(aw2, aw, part_sb[:, b, 0:1])
                lg = sp.tile([P, 1], f32, tag="lg")
                nc.vector.tensor_scalar(lg, aw2, rden, None, OP.mult)
                nc.scalar.activation(tcol[:, b:b + 1], lg, AT.Tanh,
                                     scale=0.5, bias=0.5 * cfg["bo_eff"])

            # ------------------ emission schedule ----------------------
            AHEAD_P = 4
            MERGE_I = NB

            with tc.tile_pool(name="gpool", bufs=AHEAD_P) as gp:
                # lo tables first so pass-L gathers unblock early, then
                # xrT_L (needed by the first buckets), then the rest.
                with tc.tile_pool(name="p1lp", bufs=4, space="PSUM") as lpp, \
                     tc.tile_pool(name="p1x", bufs=3) as xp, \
                     tc.tile_pool(name="p1p", bufs=2, space="PSUM") as pp, \
                     tc.tile_pool(name="p1c", bufs=3) as cvp:
                    phase1b(range(L_lo), xp, pp, cvp)
                    phase1a(xT_L_sb, xrT_L, lpp)
                    phase1b(range(L_lo, n_chunks), xp, pp, cvp)
                    phase1a(xT_H_sb, xrT_H, lpp)

                with tc.tile_pool(name="gat", bufs=2) as lp, \
                     tc.tile_pool(name="zps", bufs=3, space="PSUM") as zp, \
                     tc.tile_pool(name="sm", bufs=3) as sp, \
                     tc.tile_pool(name="dg", bufs=3) as dp, \
                     tc.tile_pool(name="aggp", bufs=2, space="PSUM") as aggp, \
                     tc.tile_pool(name="zsp", bufs=2, space="PSUM") as zsp, \
                     tc.tile_pool(name="psp", bufs=2) as psp:
                    for i in range(min(AHEAD_P, NW)):
                        if i != MERGE_I:
                            emit_gather(i, gp)
                    for i in range(NW):
                        if i == MERGE_I:
                            # merge gather must follow every partial write
                            emit_gather(MERGE_I, gp)
                        p = i + AHEAD_P
                        if p < NW and p != MERGE_I:
                            emit_gather(p, gp)
                        bucket_compute(i, lp, zp, sp, dp, aggp, zsp, psp)

                    # out = 0.5 * tanh + 0.5
                    nc.vector.tensor_scalar(out_sb, tcol, 0.5, 0.5,
                                            OP.mult, OP.add)

            nc.sync.dma_start(
                out=out_d.ap().rearrange("(b n) o -> n (b o)", n=P),
                in_=out_sb)
    nc.compile()
    return nc


# --------------------------------------------------------------------------
# Entry point
# --------------------------------------------------------------------------

def _run(inputs, trace=False, lrelu_act=True):
    from concourse.bass_utils import run_bass_kernel_spmd

    cfg, in_maps, out_nodes = _plan(**inputs)
    nc = _build(cfg, lrelu_act=lrelu_act)
    res = run_bass_kernel_spmd(nc, in_maps, core_ids=list(range(cfg["C"])),
                               trace=trace)

    N = cfg["N"]
    out = np.zeros((N, 1), dtype=np.float32)
    for c in range(cfg["C"]):
        nodes = out_nodes[c]
        ok = nodes >= 0
        out[nodes[ok], 0] = res.results[c]["out"][ok, 0]
    return out, res


def kernel(**inputs):
    return _run(inputs)[0]
